# revision 9
# baseline (speedup 1.0000x reference)
"""Multi-class DICE loss on 8 Trainium2 NeuronCores.

Reference computation (B=16, C=8, H=W=512):
    onehot = (mask[:,None] == arange(C))        # [B,C,H,W]
    num  = sum(output * onehot, axis=(2,3))     # [B,C]
    den1 = sum(output * output, axis=(2,3))     # [B,C]
    den2 = sum(onehot, axis=(2,3))              # [B,C]
    dice = 2 * (num + eps) / (den1 + den2 + eps)
    loss = 1 - sum(dice) / (B*B)

Sharding: pure data parallel over batch; each of 8 cores takes 2
samples (16 (b,c) class-tiles of [128, 2048]).

v6 layout. Measured facts from v2-v4 traces: the DMA fabric sustains
~400-430 GB/s (x stream ~44us), DVE accumulate ops are pinned to 1x
mode (2.29us/tile), and SWDGE cast-DMA runs at full read-side rate.
So x arrives as bf16 via gpsimd cast-DMA and NO DVE op accumulates:
  DVE  stt prod=(mask==c)*x, all-bf16 2x mode, no accum   (1.13us)
  DVE  ts   eq=(mask==c), bf16 4x mode, no accum          (0.69us)
  ACT  Square(x bf16), accum -> p_den1 col                (1.99us)
  ACT  u8->bf16 mask cast (per sample)
  PE   one-hot-lhsT matmuls fold prod -> psN[col, 0:512]
       and eq -> ps2[col, 0:512]; two 60/64-matmul chains
num and den2 free-dim folds are each ONE tensor_reduce [16,512] at
the end. The mask ships as uint8 (values 0..7, lossless). eq for
c == 7 is skipped: den2[7] = HW - sum(den2[0..6]) on host.
Outputs are raw partials (num [16], den1 [16], den2 [16]); dice and
the 1 - 2*sum/B^2 affine run on host in the unshard step.
"""

import os
from contextlib import ExitStack

import numpy as np

import concourse.bacc as bacc
import concourse.bass as bass
import concourse.tile as tile
from concourse import mybir
from concourse.bass_utils import run_bass_kernel_spmd

N_CORES = 8
B, C, H, W = 16, 8, 512, 512
B_LOC = B // N_CORES          # samples per core
HWPIX = H * W                 # 262144 pixels per (b, c)
P = 128                       # SBUF partitions
NCOL = HWPIX // P             # 2048 free-dim columns per class-tile
ROWS = B_LOC * C              # 16 (b, c) pairs per core
G = 4                         # max classes per x DMA group
NCHUNK = 4                    # fold matmul chunks per class-tile
CHUNK = NCOL // NCHUNK        # 512
EPS = 1e-7


_cache: dict = {}
last_results = None           # BassKernelResults of the most recent run


def _build() -> bass.Bass:
    nc = bacc.Bacc(
        "TRN2",
        target_bir_lowering=False,
        debug=False,
        num_devices=1,
    )
    f32 = mybir.dt.float32
    bf16 = mybir.dt.bfloat16
    u8 = mybir.dt.uint8

    x = nc.dram_tensor("x", [ROWS, P, NCOL], f32, kind="ExternalInput")
    m = nc.dram_tensor("m", [B_LOC, P, NCOL], u8, kind="ExternalInput")
    nd1 = nc.dram_tensor("nd1", [ROWS], f32, kind="ExternalOutput")
    nm = nc.dram_tensor("nm", [ROWS], f32, kind="ExternalOutput")
    d2 = nc.dram_tensor("d2", [ROWS], f32, kind="ExternalOutput")

    with tile.TileContext(nc) as tc, ExitStack() as ctx:
        xpool = ctx.enter_context(tc.tile_pool(name="xp", bufs=7))
        mpool = ctx.enter_context(tc.tile_pool(name="mp", bufs=2))
        mfpool = ctx.enter_context(tc.tile_pool(name="mfp", bufs=2))
        epool = ctx.enter_context(tc.tile_pool(name="ep", bufs=3))
        ppool = ctx.enter_context(tc.tile_pool(name="pp", bufs=3))
        spool = ctx.enter_context(tc.tile_pool(name="sp", bufs=2))
        acc = ctx.enter_context(tc.tile_pool(name="acc", bufs=1))
        pspool = ctx.enter_context(tc.tile_pool(name="ps", bufs=1, space="PSUM"))

        ones32 = acc.tile([P, 1], f32, tag="ones32")
        nc.vector.memset(ones32, 1.0)
        # One-hot lhsT matrices: etab[col][:, m] = 1 iff m == col. A
        # matmul with lhsT=etab[col] deposits the partition-fold of its
        # rhs into PSUM row `col` and exact zeros elsewhere (PE output
        # base-partition must be 0/32/64, so rows can't be addressed via
        # the out AP). Built on GpSimd before its DMA work starts.
        etab = []
        for col in range(ROWS):
            e = acc.tile([P, ROWS], bf16, tag=f"e{col}")
            nc.gpsimd.memset(e, 0.0)
            nc.gpsimd.memset(e[:, col : col + 1], 1.0)
            etab.append(e)

        # den1 accumulates per partition (ACT's accumulate is free).
        p_den1 = acc.tile([P, ROWS], f32, tag="p_den1")
        # num / den2: one PSUM row per (b, c), chunk-accumulated by PE.
        ps2 = pspool.tile([ROWS, CHUNK], f32, tag="ps2")
        psN = pspool.tile([ROWS, CHUNK], f32, tag="psN")

        first_n = [True]
        first_e = [True]

        for b in range(B_LOC):
            mraw = mpool.tile([P, NCOL], u8, tag="mraw")
            nc.sync.dma_start(out=mraw, in_=m[b])
            # u8 -> bf16 label copy on ACT (labels 0..7 exact); keeps
            # DVE free for the per-class stream.
            mf = mfpool.tile([P, NCOL], bf16, tag="mf")
            nc.scalar.activation(
                out=mf, in_=mraw, func=mybir.ActivationFunctionType.Copy
            )

            # den2 for classes 0..6 up-front: depends only on the mask,
            # so DVE does these while x is still streaming in. den2[7]
            # is recovered on host from sum(den2) == HWPIX.
            for c in range(C - 1):
                col = b * C + c
                eq = epool.tile([P, NCOL], bf16, tag="eq")
                nc.vector.tensor_scalar(
                    out=eq,
                    in0=mf,
                    scalar1=float(c),
                    scalar2=None,
                    op0=mybir.AluOpType.is_equal,
                )
                for j in range(NCHUNK):
                    nc.tensor.matmul(
                        out=ps2[:, :],
                        lhsT=etab[col],
                        rhs=eq[:, j * CHUNK : (j + 1) * CHUNK],
                        start=first_e[0],
                        stop=(b == B_LOC - 1 and c == C - 2 and j == NCHUNK - 1),
                    )
                    first_e[0] = False

            # x stream: SWDGE cast-DMA f32->bf16, staircased so compute
            # starts after ~1 MiB and the post-stream tail is short.
            groups = [1, 3, G] if b == 0 else [G, 3, 1]
            c0 = 0
            for gsz in groups:
                xt = xpool.tile([P, G, NCOL], bf16, tag="xt")
                nc.gpsimd.dma_start(
                    out=xt[:, 0:gsz, :],
                    in_=x[b * C + c0 : b * C + c0 + gsz].transpose([1, 0, 2]),
                )
                for i in range(gsz):
                    c = c0 + i
                    col = b * C + c
                    # num: prod = (mask == c) * x in bf16 (2x mode), PE
                    # folds partitions into psN row `col`.
                    prod = ppool.tile([P, NCOL], bf16, tag="prod")
                    nc.vector.scalar_tensor_tensor(
                        out=prod,
                        in0=mf,
                        scalar=float(c),
                        in1=xt[:, i, :],
                        op0=mybir.AluOpType.is_equal,
                        op1=mybir.AluOpType.mult,
                    )
                    for j in range(NCHUNK):
                        nc.tensor.matmul(
                            out=psN[:, :],
                            lhsT=etab[col],
                            rhs=prod[:, j * CHUNK : (j + 1) * CHUNK],
                            start=first_n[0],
                            stop=(b == B_LOC - 1
                                  and c == C - 1 and j == NCHUNK - 1),
                        )
                        first_n[0] = False
                    # den1: x^2, accumulated per partition on ACT
                    sjunk = spool.tile([P, NCOL], mybir.dt.float8e4, tag="ja")
                    nc.scalar.activation(
                        out=sjunk,
                        in_=xt[:, i, :],
                        func=mybir.ActivationFunctionType.Square,
                        accum_out=p_den1[:, col : col + 1],
                    )
                c0 += gsz

        # den1 fold: [128, 16] -> PSUM [1, 16] -> SBUF -> DRAM.
        ps1 = pspool.tile([1, ROWS], f32, tag="ps1")
        nc.tensor.matmul(out=ps1[:], lhsT=ones32, rhs=p_den1[:], start=True, stop=True)
        nd1s = acc.tile([1, ROWS], f32, tag="nd1s")
        nc.vector.tensor_copy(out=nd1s, in_=ps1[:])
        nc.sync.dma_start(out=nd1[:], in_=nd1s)

        # num / den2: fold the chunk dim of all 16 rows at once.
        nmcol = acc.tile([ROWS, 1], f32, tag="nmcol")
        nc.vector.tensor_reduce(
            out=nmcol, in_=psN[:], axis=mybir.AxisListType.X, op=mybir.AluOpType.add
        )
        nc.sync.dma_start(out=nm[:], in_=nmcol)
        d2col = acc.tile([ROWS, 1], f32, tag="d2col")
        nc.vector.tensor_reduce(
            out=d2col, in_=ps2[:], axis=mybir.AxisListType.X, op=mybir.AluOpType.add
        )
        nc.sync.dma_start(out=d2[:], in_=d2col)

    nc.compile()
    return nc


def _get(mask64: bool = False) -> bass.Bass:
    if "k" not in _cache:
        _cache["k"] = _build()
    return _cache["k"]


def make_in_maps(output: np.ndarray, mask: np.ndarray, mask64: bool = False):
    # Labels are 0..7: ship the mask as uint8 (lossless) to cut its DMA 4x.
    m8 = mask.astype(np.uint8)
    in_maps = []
    for i in range(N_CORES):
        xs = output[i * B_LOC : (i + 1) * B_LOC].reshape(ROWS, P, NCOL)
        ms = m8[i * B_LOC : (i + 1) * B_LOC].reshape(B_LOC, P, NCOL)
        in_maps.append(
            {"x": np.ascontiguousarray(xs), "m": np.ascontiguousarray(ms)}
        )
    return in_maps


def kernel(output: np.ndarray, mask: np.ndarray) -> np.ndarray:
    global last_results
    output = np.ascontiguousarray(np.asarray(output, dtype=np.float32))
    mask = np.asarray(mask)
    assert output.shape == (B, C, H, W), output.shape
    assert mask.shape == (B, H, W), mask.shape

    nc = _get()
    in_maps = make_in_maps(output, mask)
    last_results = run_bass_kernel_spmd(
        nc,
        in_maps,
        list(range(N_CORES)),
        trace=bool(os.environ.get("DICE_TRACE")),
    )
    # Unshard: dice over the gathered per-(b,c) partials, then the
    # 1 - 2*sum/B^2 affine. den2[c=7] comes from sum(den2) == HWPIX.
    total = 0.0
    for r in last_results.results:
        num = np.asarray(r["nm"], dtype=np.float64).reshape(B_LOC, C)
        den1 = np.asarray(r["nd1"], dtype=np.float64).reshape(B_LOC, C)
        den2 = np.asarray(r["d2"], dtype=np.float64).reshape(B_LOC, C).copy()
        den2[:, C - 1] = HWPIX - den2[:, : C - 1].sum(axis=1)
        total += float(np.sum((num + EPS) / (den1 + den2 + EPS)))
    loss = 1.0 - 2.0 * total / (B * B)
    return np.float32(loss).reshape(())


# revision 10
# speedup vs baseline: 1.1171x; 1.1171x over previous
"""Multi-class DICE loss on 8 Trainium2 NeuronCores.

Reference computation (B=16, C=8, H=W=512):
    onehot = (mask[:,None] == arange(C))        # [B,C,H,W]
    num  = sum(output * onehot, axis=(2,3))     # [B,C]
    den1 = sum(output * output, axis=(2,3))     # [B,C]
    den2 = sum(onehot, axis=(2,3))              # [B,C]
    dice = 2 * (num + eps) / (den1 + den2 + eps)
    loss = 1 - sum(dice) / (B*B)

Sharding: pure data parallel over batch; each of 8 cores takes 2
samples (16 (b,c) class-tiles of [128, 2048]).

v7 layout (v4 + stream/slot tuning). Measured: DMA fabric sustains
~420 GB/s; DVE accumulate ops are pinned to 1x (2.29us/tile) and
scalar_tensor_tensor has no 2x uop, so num stays on the DVE accum
path and the engines balance as:
  DVE  scalar_tensor_tensor (mask==c)*x f32, accum -> p_num col
  DVE  tensor_scalar eq=(mask==c) bf16 4x, NO accum (c<7 only;
       den2[7] = HWPIX - sum on host), emitted before the x loop so
       they fill DVE's head gap while x streams.
  ACT  Square x f32, accum -> p_den1 col; u8->bf16 mask casts
  PE   one-hot-lhsT matmuls fold eq partitions -> ps2[col, 0:512]
Every x group has its OWN resident SBUF slot (per-size pools), so the
SP queue issues the whole 17.3 MB stream up-front and never waits on
compute to release buffers. The mask ships as uint8 (lossless).
Outputs are raw partials (num|den1 [32], den2 [16]); dice and the
1 - 2*sum/B^2 affine run on host in the unshard step.
"""

import os
from contextlib import ExitStack

import numpy as np

import concourse.bacc as bacc
import concourse.bass as bass
import concourse.tile as tile
from concourse import mybir
from concourse.bass_utils import run_bass_kernel_spmd

N_CORES = 8
B, C, H, W = 16, 8, 512, 512
B_LOC = B // N_CORES          # samples per core
HWPIX = H * W                 # 262144 pixels per (b, c)
P = 128                       # SBUF partitions
NCOL = HWPIX // P             # 2048 free-dim columns per class-tile
ROWS = B_LOC * C              # 16 (b, c) pairs per core
NCHUNK = 4                    # eq matmul chunks per class-tile
CHUNK = NCOL // NCHUNK        # 512
EPS = 1e-7


_cache: dict = {}
last_results = None           # BassKernelResults of the most recent run


def _build() -> bass.Bass:
    nc = bacc.Bacc(
        "TRN2",
        target_bir_lowering=False,
        debug=False,
        num_devices=1,
    )
    f32 = mybir.dt.float32
    bf16 = mybir.dt.bfloat16
    u8 = mybir.dt.uint8

    x = nc.dram_tensor("x", [ROWS, P, NCOL], f32, kind="ExternalInput")
    m = nc.dram_tensor("m", [B_LOC, P, NCOL], u8, kind="ExternalInput")
    nd = nc.dram_tensor("nd", [2 * ROWS], f32, kind="ExternalOutput")
    d2 = nc.dram_tensor("d2", [ROWS], f32, kind="ExternalOutput")

    with tile.TileContext(nc) as tc, ExitStack() as ctx:
        # One resident slot per x group: the whole stream can be in
        # flight with zero buffer-release back-pressure.
        xp1 = ctx.enter_context(tc.tile_pool(name="xp1", bufs=2))
        xp3 = ctx.enter_context(tc.tile_pool(name="xp3", bufs=2))
        xp4 = ctx.enter_context(tc.tile_pool(name="xp4", bufs=3))
        xpools = {1: xp1, 3: xp3, 4: xp4}
        mpool = ctx.enter_context(tc.tile_pool(name="mp", bufs=2))
        mfpool = ctx.enter_context(tc.tile_pool(name="mfp", bufs=2))
        epool = ctx.enter_context(tc.tile_pool(name="ep", bufs=3))
        jpool = ctx.enter_context(tc.tile_pool(name="jp", bufs=1))
        spool = ctx.enter_context(tc.tile_pool(name="sp", bufs=1))
        acc = ctx.enter_context(tc.tile_pool(name="acc", bufs=1))
        pspool = ctx.enter_context(tc.tile_pool(name="ps", bufs=1, space="PSUM"))

        ones32 = acc.tile([P, 1], f32, tag="ones32")
        nc.vector.memset(ones32, 1.0)
        # One-hot lhsT matrices: etab[col][:, m] = 1 iff m == col. A
        # matmul with lhsT=etab[col] deposits the partition-fold of its
        # rhs into PSUM row `col` and exact zeros elsewhere (PE output
        # base-partition must be 0/32/64, so rows can't be addressed via
        # the out AP). Built on GpSimd, which is otherwise idle.
        etab = []
        for col in range(ROWS):
            e = acc.tile([P, ROWS], bf16, tag=f"e{col}")
            nc.gpsimd.memset(e, 0.0)
            nc.gpsimd.memset(e[:, col : col + 1], 1.0)
            etab.append(e)

        # Per-partition partial sums, one column per (b, c) pair.
        # Separate tiles per writing engine so DVE and ACT accumulator
        # writes never cross-serialize.
        p_num = acc.tile([P, ROWS], f32, tag="p_num")
        p_den1 = acc.tile([P, ROWS], f32, tag="p_den1")
        # den2: one PSUM row per (b, c), chunk-accumulated by PE.
        ps2 = pspool.tile([ROWS, CHUNK], f32, tag="ps2")

        first_e = [True]
        for b in range(B_LOC):
            mraw = mpool.tile([P, NCOL], u8, tag="mraw")
            nc.sync.dma_start(out=mraw, in_=m[b])
            # u8 -> bf16 label copy on ACT (labels 0..7 exact); keeps
            # DVE free for the per-class stt stream.
            mf = mfpool.tile([P, NCOL], bf16, tag="mf")
            nc.scalar.activation(
                out=mf, in_=mraw, func=mybir.ActivationFunctionType.Copy
            )

            # den2 for classes 0..6 up-front: depends only on the mask,
            # so DVE does these while x is still streaming in. den2[7]
            # is recovered on host from sum(den2) == HWPIX.
            for c in range(C - 1):
                col = b * C + c
                eq = epool.tile([P, NCOL], bf16, tag="eq")
                nc.vector.tensor_scalar(
                    out=eq,
                    in0=mf,
                    scalar1=float(c),
                    scalar2=None,
                    op0=mybir.AluOpType.is_equal,
                )
                for j in range(NCHUNK):
                    nc.tensor.matmul(
                        out=ps2[:, :],
                        lhsT=etab[col],
                        rhs=eq[:, j * CHUNK : (j + 1) * CHUNK],
                        start=first_e[0],
                        stop=(b == B_LOC - 1 and c == C - 2 and j == NCHUNK - 1),
                    )
                    first_e[0] = False

            # x stream: staircased so compute starts after 1 MiB and the
            # post-stream compute tail is a single class.
            groups = [1, 3, 4] if b == 0 else [4, 3, 1]
            c0 = 0
            for gsz in groups:
                xt = xpools[gsz].tile([P, gsz, NCOL], f32, tag=f"xt{gsz}")
                nc.sync.dma_start(
                    out=xt[:, 0:gsz, :],
                    in_=x[b * C + c0 : b * C + c0 + gsz].transpose([1, 0, 2]),
                )
                for i in range(gsz):
                    c = c0 + i
                    col = b * C + c
                    # num partial: (mask == c) * x, accumulated per partition
                    junk = jpool.tile([P, NCOL], mybir.dt.float8e4, tag="jd")
                    nc.vector.scalar_tensor_tensor(
                        out=junk,
                        in0=mf,
                        scalar=float(c),
                        in1=xt[:, i, :],
                        op0=mybir.AluOpType.is_equal,
                        op1=mybir.AluOpType.mult,
                        accum_out=p_num[:, col : col + 1],
                    )
                    # den1 partial: x^2, accumulated per partition
                    sjunk = spool.tile([P, NCOL], mybir.dt.float8e4, tag="ja")
                    nc.scalar.activation(
                        out=sjunk,
                        in_=xt[:, i, :],
                        func=mybir.ActivationFunctionType.Square,
                        accum_out=p_den1[:, col : col + 1],
                    )
                c0 += gsz

        # Fold partition dim of num/den1: [128, 16] -> PSUM [1, 32].
        ps = pspool.tile([1, 2 * ROWS], f32, tag="ps")
        nc.tensor.matmul(out=ps[:, 0:ROWS], lhsT=ones32, rhs=p_num[:], start=True, stop=True)
        nc.tensor.matmul(out=ps[:, ROWS:], lhsT=ones32, rhs=p_den1[:], start=True, stop=True)
        nds = acc.tile([1, 2 * ROWS], f32, tag="nds")
        nc.vector.tensor_copy(out=nds, in_=ps[:])
        nc.sync.dma_start(out=nd[:], in_=nds)

        # den2: fold the chunk dim of all 16 rows at once.
        d2col = acc.tile([ROWS, 1], f32, tag="d2col")
        nc.vector.tensor_reduce(
            out=d2col, in_=ps2[:], axis=mybir.AxisListType.X, op=mybir.AluOpType.add
        )
        nc.sync.dma_start(out=d2[:], in_=d2col)

    nc.compile()
    return nc


def _get(mask64: bool = False) -> bass.Bass:
    if "k" not in _cache:
        _cache["k"] = _build()
    return _cache["k"]


def make_in_maps(output: np.ndarray, mask: np.ndarray, mask64: bool = False):
    # Labels are 0..7: ship the mask as uint8 (lossless) to cut its DMA 4x.
    m8 = mask.astype(np.uint8)
    in_maps = []
    for i in range(N_CORES):
        xs = output[i * B_LOC : (i + 1) * B_LOC].reshape(ROWS, P, NCOL)
        ms = m8[i * B_LOC : (i + 1) * B_LOC].reshape(B_LOC, P, NCOL)
        in_maps.append(
            {"x": np.ascontiguousarray(xs), "m": np.ascontiguousarray(ms)}
        )
    return in_maps


def kernel(output: np.ndarray, mask: np.ndarray) -> np.ndarray:
    global last_results
    output = np.ascontiguousarray(np.asarray(output, dtype=np.float32))
    mask = np.asarray(mask)
    assert output.shape == (B, C, H, W), output.shape
    assert mask.shape == (B, H, W), mask.shape

    nc = _get()
    in_maps = make_in_maps(output, mask)
    last_results = run_bass_kernel_spmd(
        nc,
        in_maps,
        list(range(N_CORES)),
        trace=bool(os.environ.get("DICE_TRACE")),
    )
    # Unshard: dice over the gathered per-(b,c) partials, then the
    # 1 - 2*sum/B^2 affine. den2[c=7] comes from sum(den2) == HWPIX.
    total = 0.0
    for r in last_results.results:
        nd_ = np.asarray(r["nd"], dtype=np.float64).reshape(2, B_LOC, C)
        num, den1 = nd_[0], nd_[1]
        den2 = np.asarray(r["d2"], dtype=np.float64).reshape(B_LOC, C).copy()
        den2[:, C - 1] = HWPIX - den2[:, : C - 1].sum(axis=1)
        total += float(np.sum((num + EPS) / (den1 + den2 + EPS)))
    loss = 1.0 - 2.0 * total / (B * B)
    return np.float32(loss).reshape(())


# revision 12
# speedup vs baseline: 1.1992x; 1.0735x over previous
"""Multi-class DICE loss on 8 Trainium2 NeuronCores.

Reference computation (B=16, C=8, H=W=512):
    onehot = (mask[:,None] == arange(C))        # [B,C,H,W]
    num  = sum(output * onehot, axis=(2,3))     # [B,C]
    den1 = sum(output * output, axis=(2,3))     # [B,C]
    den2 = sum(onehot, axis=(2,3))              # [B,C]
    dice = 2 * (num + eps) / (den1 + den2 + eps)
    loss = 1 - sum(dice) / (B*B)

Sharding: pure data parallel over batch; each of 8 cores takes 2
samples (16 (b,c) class-tiles of [128, 2048]).

v7 layout (v4 + stream/slot tuning). Measured: DMA fabric sustains
~420 GB/s; DVE accumulate ops are pinned to 1x (2.29us/tile) and
scalar_tensor_tensor has no 2x uop, so num stays on the DVE accum
path and the engines balance as:
  DVE  scalar_tensor_tensor (mask==c)*x f32, accum -> p_num col
  DVE  tensor_scalar eq=(mask==c) bf16 4x, NO accum (c<7 only;
       den2[7] = HWPIX - sum on host), emitted before the x loop so
       they fill DVE's head gap while x streams.
  ACT  Square x f32, accum -> p_den1 col; u8->bf16 mask casts
  PE   one-hot-lhsT matmuls fold eq partitions -> ps2[col, 0:512]
Every x group has its OWN resident SBUF slot (per-size pools), so the
SP queue issues the whole 17.3 MB stream up-front and never waits on
compute to release buffers. The mask ships as uint8 (lossless).
Outputs are raw partials (num|den1 [32], den2 [16]); dice and the
1 - 2*sum/B^2 affine run on host in the unshard step.
"""

import os
from contextlib import ExitStack

import numpy as np

import concourse.bacc as bacc
import concourse.bass as bass
import concourse.tile as tile
from concourse import mybir
from concourse.bass_utils import run_bass_kernel_spmd

N_CORES = 8
B, C, H, W = 16, 8, 512, 512
B_LOC = B // N_CORES          # samples per core
HWPIX = H * W                 # 262144 pixels per (b, c)
P = 128                       # SBUF partitions
NCOL = HWPIX // P             # 2048 free-dim columns per class-tile
ROWS = B_LOC * C              # 16 (b, c) pairs per core
NCHUNK = 4                    # eq matmul chunks per class-tile
CHUNK = NCOL // NCHUNK        # 512
EPS = 1e-7


_cache: dict = {}
last_results = None           # BassKernelResults of the most recent run


def _build() -> bass.Bass:
    nc = bacc.Bacc(
        "TRN2",
        target_bir_lowering=False,
        debug=False,
        num_devices=1,
    )
    f32 = mybir.dt.float32
    bf16 = mybir.dt.bfloat16
    u8 = mybir.dt.uint8

    x = nc.dram_tensor("x", [ROWS, P, NCOL], f32, kind="ExternalInput")
    m = nc.dram_tensor("m", [B_LOC, P, NCOL], u8, kind="ExternalInput")
    nd = nc.dram_tensor("nd", [2 * ROWS], f32, kind="ExternalOutput")
    d2 = nc.dram_tensor("d2", [ROWS], f32, kind="ExternalOutput")

    with tile.TileContext(nc) as tc, ExitStack() as ctx:
        # One resident slot per x group: the whole stream can be in
        # flight with zero buffer-release back-pressure.
        xp1 = ctx.enter_context(tc.tile_pool(name="xp1", bufs=2))
        xp3 = ctx.enter_context(tc.tile_pool(name="xp3", bufs=2))
        xp4 = ctx.enter_context(tc.tile_pool(name="xp4", bufs=3))
        xpools = {1: xp1, 3: xp3, 4: xp4}
        mpool = ctx.enter_context(tc.tile_pool(name="mp", bufs=2))
        mfpool = ctx.enter_context(tc.tile_pool(name="mfp", bufs=2))
        epool = ctx.enter_context(tc.tile_pool(name="ep", bufs=5))
        jpool = ctx.enter_context(tc.tile_pool(name="jp", bufs=1))
        spool = ctx.enter_context(tc.tile_pool(name="sp", bufs=1))
        acc = ctx.enter_context(tc.tile_pool(name="acc", bufs=1))
        pspool = ctx.enter_context(tc.tile_pool(name="ps", bufs=1, space="PSUM"))

        ones32 = acc.tile([P, 1], f32, tag="ones32")
        nc.vector.memset(ones32, 1.0)
        # One-hot lhsT matrices: etab[col][:, m] = 1 iff m == col. A
        # matmul with lhsT=etab[col] deposits the partition-fold of its
        # rhs into PSUM row `col` and exact zeros elsewhere (PE output
        # base-partition must be 0/32/64, so rows can't be addressed via
        # the out AP). Built on GpSimd, which is otherwise idle.
        etab = []
        for col in range(ROWS):
            e = acc.tile([P, ROWS], bf16, tag=f"e{col}")
            nc.gpsimd.memset(e, 0.0)
            nc.gpsimd.memset(e[:, col : col + 1], 1.0)
            etab.append(e)

        # Per-partition partial sums, one column per (b, c) pair.
        # Separate tiles per writing engine so DVE and ACT accumulator
        # writes never cross-serialize.
        p_num = acc.tile([P, ROWS], f32, tag="p_num")
        p_den1 = acc.tile([P, ROWS], f32, tag="p_den1")
        # den2: one PSUM row per (b, c), chunk-accumulated by PE.
        ps2 = pspool.tile([ROWS, CHUNK], f32, tag="ps2")

        # Both masks first: they are tiny (0.25 MiB each) and everything
        # mask-derived (casts, eq tiles) should be off the critical path
        # before the 16 MiB x stream monopolizes the queue.
        mfs = []
        for b in range(B_LOC):
            mraw = mpool.tile([P, NCOL], u8, tag="mraw")
            nc.sync.dma_start(out=mraw, in_=m[b])
            # u8 -> bf16 label copy on ACT (labels 0..7 exact); keeps
            # DVE free for the per-class stt stream.
            mf = mfpool.tile([P, NCOL], bf16, tag="mf")
            nc.scalar.activation(
                out=mf, in_=mraw, func=mybir.ActivationFunctionType.Copy
            )
            mfs.append(mf)

        # den2 for classes 0..6 of both samples up-front: depends only
        # on the masks, so DVE does these while x is still streaming.
        # den2[7] is recovered on host from sum(den2) == HWPIX.
        first_e = [True]
        for b in range(B_LOC):
            for c in range(C - 1):
                col = b * C + c
                eq = epool.tile([P, NCOL], bf16, tag="eq")
                nc.vector.tensor_scalar(
                    out=eq,
                    in0=mfs[b],
                    scalar1=float(c),
                    scalar2=None,
                    op0=mybir.AluOpType.is_equal,
                )
                for j in range(NCHUNK):
                    nc.tensor.matmul(
                        out=ps2[:, :],
                        lhsT=etab[col],
                        rhs=eq[:, j * CHUNK : (j + 1) * CHUNK],
                        start=first_e[0],
                        stop=(b == B_LOC - 1 and c == C - 2 and j == NCHUNK - 1),
                    )
                    first_e[0] = False

        for b in range(B_LOC):
            mf = mfs[b]
            # x stream: staircased so compute starts after 1 MiB and the
            # post-stream compute tail is a single class.
            groups = [1, 3, 4] if b == 0 else [4, 3, 1]
            c0 = 0
            for gsz in groups:
                xt = xpools[gsz].tile([P, gsz, NCOL], f32, tag=f"xt{gsz}")
                nc.sync.dma_start(
                    out=xt[:, 0:gsz, :],
                    in_=x[b * C + c0 : b * C + c0 + gsz].transpose([1, 0, 2]),
                )
                for i in range(gsz):
                    c = c0 + i
                    col = b * C + c
                    # num partial: (mask == c) * x, accumulated per partition
                    junk = jpool.tile([P, NCOL], mybir.dt.float8e4, tag="jd")
                    nc.vector.scalar_tensor_tensor(
                        out=junk,
                        in0=mf,
                        scalar=float(c),
                        in1=xt[:, i, :],
                        op0=mybir.AluOpType.is_equal,
                        op1=mybir.AluOpType.mult,
                        accum_out=p_num[:, col : col + 1],
                    )
                    # den1 partial: x^2, accumulated per partition
                    sjunk = spool.tile([P, NCOL], mybir.dt.float8e4, tag="ja")
                    nc.scalar.activation(
                        out=sjunk,
                        in_=xt[:, i, :],
                        func=mybir.ActivationFunctionType.Square,
                        accum_out=p_den1[:, col : col + 1],
                    )
                c0 += gsz

        # Fold partition dim of num/den1: [128, 16] -> PSUM [1, 32].
        ps = pspool.tile([1, 2 * ROWS], f32, tag="ps")
        nc.tensor.matmul(out=ps[:, 0:ROWS], lhsT=ones32, rhs=p_num[:], start=True, stop=True)
        nc.tensor.matmul(out=ps[:, ROWS:], lhsT=ones32, rhs=p_den1[:], start=True, stop=True)
        nds = acc.tile([1, 2 * ROWS], f32, tag="nds")
        nc.vector.tensor_copy(out=nds, in_=ps[:])
        nc.sync.dma_start(out=nd[:], in_=nds)

        # den2: fold the chunk dim of all 16 rows at once.
        d2col = acc.tile([ROWS, 1], f32, tag="d2col")
        nc.vector.tensor_reduce(
            out=d2col, in_=ps2[:], axis=mybir.AxisListType.X, op=mybir.AluOpType.add
        )
        nc.sync.dma_start(out=d2[:], in_=d2col)

    nc.compile()
    return nc


def _get(mask64: bool = False) -> bass.Bass:
    if "k" not in _cache:
        _cache["k"] = _build()
    return _cache["k"]


def make_in_maps(output: np.ndarray, mask: np.ndarray, mask64: bool = False):
    # Labels are 0..7: ship the mask as uint8 (lossless) to cut its DMA 4x.
    m8 = mask.astype(np.uint8)
    in_maps = []
    for i in range(N_CORES):
        xs = output[i * B_LOC : (i + 1) * B_LOC].reshape(ROWS, P, NCOL)
        ms = m8[i * B_LOC : (i + 1) * B_LOC].reshape(B_LOC, P, NCOL)
        in_maps.append(
            {"x": np.ascontiguousarray(xs), "m": np.ascontiguousarray(ms)}
        )
    return in_maps


def kernel(output: np.ndarray, mask: np.ndarray) -> np.ndarray:
    global last_results
    output = np.ascontiguousarray(np.asarray(output, dtype=np.float32))
    mask = np.asarray(mask)
    assert output.shape == (B, C, H, W), output.shape
    assert mask.shape == (B, H, W), mask.shape

    nc = _get()
    in_maps = make_in_maps(output, mask)
    last_results = run_bass_kernel_spmd(
        nc,
        in_maps,
        list(range(N_CORES)),
        trace=bool(os.environ.get("DICE_TRACE")),
    )
    # Unshard: dice over the gathered per-(b,c) partials, then the
    # 1 - 2*sum/B^2 affine. den2[c=7] comes from sum(den2) == HWPIX.
    total = 0.0
    for r in last_results.results:
        nd_ = np.asarray(r["nd"], dtype=np.float64).reshape(2, B_LOC, C)
        num, den1 = nd_[0], nd_[1]
        den2 = np.asarray(r["d2"], dtype=np.float64).reshape(B_LOC, C).copy()
        den2[:, C - 1] = HWPIX - den2[:, : C - 1].sum(axis=1)
        total += float(np.sum((num + EPS) / (den1 + den2 + EPS)))
    loss = 1.0 - 2.0 * total / (B * B)
    return np.float32(loss).reshape(())
